# revision 2
# baseline (speedup 1.0000x reference)
# nn_AttentionPairBiasPairformerDeepspeed — 8-core Bass/Tile kernel.
# Shapes hardcoded: B=1, L=768, c_a=384, c_pair=128, H=16, c=24.
# Sharding: i-axis split 8 ways (96 rows/core); weights replicated.
# Host does layout/cast/weight-folding only; all tensor math on device.
import sys as _sys

for _p in ("/opt/trn_rl_repo", "/root/problem"):
    if _p not in _sys.path:
        _sys.path.insert(0, _p)

import numpy as np

L = 768
NCORES = 8
LS = L // NCORES  # 96

_CACHE = {}


def _get_nc():
    if "nc" not in _CACHE:
        import bass_kernel as bk
        _CACHE["nc"] = bk.build(L=L, LS=LS, use_xbar=True, num_devices=NCORES)
    return _CACHE["nc"]


def kernel(A_I, Z_II, Beta_II, Wq, Wk, Wv, Wg, Wb, Wa, ln0_g, ln0_b, ln1_g, ln1_b):
    import bass_kernel as bk
    from concourse.bass_utils import run_bass_kernel_spmd

    inputs = dict(A_I=A_I, Z_II=Z_II, Beta_II=Beta_II, Wq=Wq, Wk=Wk, Wv=Wv,
                  Wg=Wg, Wb=Wb, Wa=Wa, ln0_g=ln0_g, ln0_b=ln0_b,
                  ln1_g=ln1_g, ln1_b=ln1_b)
    wts = bk.prep_weights(inputs, L=L, LS=LS)
    in_maps = [{**bk.prep_core_inputs(inputs, c, ncores=NCORES, L=L), **wts}
               for c in range(NCORES)]
    nc = _get_nc()
    res = run_bass_kernel_spmd(nc, in_maps, core_ids=list(range(NCORES)))
    out = np.concatenate([res.results[c]["out"] for c in range(NCORES)], axis=0)
    return out.reshape(1, L, 384).astype(np.float32)
